# revision 25
# baseline (speedup 1.0000x reference)
"""Gaussian kernel matrix on 8 Trainium2 NeuronCores — host-stats fp8 GEMM.

out = exp(-d2 / (2*sigma^2)),  d2[i,j] = ||x_i||^2 + ||x_j||^2 - 2 x_i.x_j,
sigma^2 = mean(d2) = 2*(mean(sq) - ||mean(X)||^2).

Strategy v4:
- Symmetry: core c computes rows [c*512,(c+1)*512) x a wrapped column window
  of 2560 cols starting at c*512 (5 of 8 j-blocks); the host mirrors the
  remaining blocks by transposition. The diagonal j-block is computed only
  for cols >= t*128 per row-tile (upper triangle at 128-granularity); the
  host mirrors the rest.
- GEMM in fp8 e4m3 DoubleRowSwInterleave (K=256/matmul); X quantized on the
  host, kernel computes the exact Gaussian kernel of the quantized points.
- ALL statistics on the host (O(N*D)): sq_j, exact sigma^2 from unquantized
  X, q_j = -(sq_j-512)/2 baked into the sacrificed contraction row (logical
  dim 480) of wj; bias_i/scale shipped as a [P,5] f32 input consumed by the
  fused ACT epilogue out = Exp(scale*G + bias_i).
- PE p-state: max clock needs ~3us of continuous PE busy; idle gaps reset
  it. Warmup matmuls over raw (uninitialized) SBUF bridge from the PE
  preamble until the first wj chunk lands.
- Epilogue: narrow (triangle) group PSUM->ACT first for an early start,
  then 8 wide [128,1024] 2-bank PSUM tiles, each one Exp pass (amortizes
  the ~250ns fixed ACT PSUM-access cost).
- DMA: all DRAM tensors laid out so every transfer is contiguous in DRAM;
  few large transfers (each issue costs ~0.65us of engine time), placed in
  consumption order across the three DMA queues (SP/sync,
  Activation/scalar, Pool/gpsimd); outputs alternate sync/gpsimd with the
  final tile split onto sync+scalar (scalar is free after its last ACT).
"""
import numpy as np
import sys

sys.path.insert(0, "/opt/trn_rl_repo")
from concourse import bass, tile, mybir  # noqa: E402
from concourse.bass_utils import run_bass_kernel_spmd  # noqa: E402

N, D, NCORES = 4096, 512, 8
RPC = 512                  # output rows per core
P = 128                    # partitions
KT = 2                     # DoubleRow k-tiles (256 contraction rows each)
JB = 512                   # j-block width
W = 5                      # window j-blocks per core
WIN = W * JB               # 2560 window columns
NT = RPC // P              # 4 row-tiles per core
NWARM = 19                 # PE warmup matmuls (~215ns each)
f32 = mybir.dt.float32
f16 = mybir.dt.float16
bf16 = mybir.dt.bfloat16
fp8 = mybir.dt.float8e4
ACTF = mybir.ActivationFunctionType
DRS = mybir.MatmulPerfMode.DoubleRowSwInterleave

# wide groups: (window col offset, width); each pairs 2 j-blocks into a
# 2-bank PSUM tile. The diagonal block (cols 0:512) is the narrow group.
WIDE = [(512, 1024), (1536, 1024)]


def _split_waits(nc, max_waits=1):
    """walrus in this image encodes at most one sync-wait per instruction;
    split extras into single-wait NOPs placed just before the instruction."""
    for fn in nc.m.functions:
        for bb in fn.blocks:
            out = []
            for inst in bb.instructions:
                si = inst.sync_info
                if si and si.on_wait and len(si.on_wait) > max_waits:
                    waits = list(si.on_wait)
                    extra, keep = waits[:-max_waits], waits[-max_waits:]
                    for j, w in enumerate(extra):
                        out.append(mybir.InstNoOp(
                            name=f"{inst.name}-ws{j}", engine=inst.engine,
                            sync_info=mybir.SyncInfo(on_wait=[w], on_update=[])))
                    si.on_wait = keep
                out.append(inst)
            bb.instructions = out


def build():
    nc = bass.Bass()
    wj_in = nc.dram_tensor("wj", [KT, P, 2, W, JB], fp8, kind="ExternalInput")
    xtc_in = nc.dram_tensor("xtc", [P, KT, RPC, 2], fp8, kind="ExternalInput")
    stat_in = nc.dram_tensor("stat", [P, NT + 1], f32, kind="ExternalInput")
    out_d = nc.dram_tensor("out", [NT, W, P, JB], bf16, kind="ExternalOutput")

    with tile.TileContext(nc) as tc:
        with (
            tc.tile_pool(name="xt", bufs=1) as xt_pool,
            tc.tile_pool(name="ot", bufs=1) as ot_pool,
            tc.tile_pool(name="small", bufs=1) as small_pool,
            tc.tile_pool(name="g", bufs=3, space="PSUM") as g_pool,
            tc.tile_pool(name="wup", bufs=1, space="PSUM") as wup_pool,
        ):
            xtcT = xt_pool.tile([P, KT, RPC, 2], fp8, name="xtc", tag="xtc")
            xtc = [xtcT[:, k] for k in range(KT)]
            wj = [xt_pool.tile([P, 2, W, JB], fp8, name=f"wj{k}", tag=f"wj{k}")
                  for k in range(KT)]
            stat_sb = small_pool.tile([P, NT + 1], f32, tag="stat")

            def wjd(q, k, w0, w1):
                q.dma_start(wj[k][:, :, w0:w1, :], wj_in[k][:, :, w0:w1, :])

            # Transfers in consumption order across the three queues (HBM
            # read BW ~360 GB/s shared, so the concurrent set at any time
            # should be the next-needed chunks). DMA completion semaphores
            # cost ~1us, queues spin up ~1.2us after the first issue, and
            # the gpsimd (SWDGE) queue starts ~1.5us late — it gets only
            # chunks needed in the wide phase.
            wjd(nc.scalar, 0, 0, 1)
            nc.scalar.dma_start(xtcT[:, 1], xtc_in[:, 1])
            nc.scalar.dma_start(stat_sb[:], stat_in[:, :])
            wjd(nc.scalar, 0, 1, 2)
            wjd(nc.scalar, 0, 3, 4)

            nc.sync.dma_start(xtcT[:, 0], xtc_in[:, 0])
            wjd(nc.sync, 1, 0, 1)
            wjd(nc.sync, 0, 2, 3)
            wjd(nc.sync, 1, 3, 4)

            wjd(nc.gpsimd, 1, 1, 2)
            wjd(nc.gpsimd, 1, 2, 3)
            wjd(nc.gpsimd, 0, 4, 5)
            wjd(nc.gpsimd, 1, 4, 5)

            # Exp ACT table load (~1.3us) during DMA dead time.
            warm_f = small_pool.tile([1, 16], f32, tag="warm_f")
            nc.vector.memset(warm_f[:], 1.0)
            warm_sb = small_pool.tile([1, 16], f32, tag="warm_sb")
            nc.scalar.activation(warm_sb[:], warm_f[:], ACTF.Exp)

            # PE p-state warmup over raw SBUF (no deps, garbage values into
            # a scratch PSUM bank).
            wstat = nc.alloc_sbuf_tensor("wup_stat", [P, P, 2], fp8).ap()
            wmov = nc.alloc_sbuf_tensor("wup_mov", [P, 2, 256], fp8).ap()
            wp = wup_pool.tile([P, 256], f32, tag="wup")
            for i in range(NWARM):
                nc.tensor.matmul(wp[:], wstat, wmov,
                                 start=True, stop=True, perf_mode=DRS)

            def mms(gp, t, coff, width):
                for s in range(-(-width // JB)):
                    a, b = s * JB, min((s + 1) * JB, width)
                    w, cl = (coff + a) // JB, (coff + a) % JB
                    for k in range(KT):
                        nc.tensor.matmul(
                            gp[:, a:b],
                            xtc[k][:, t * P:(t + 1) * P, :],
                            wj[k][:, :, w, cl:cl + (b - a)],
                            start=(k == 0), stop=(k == KT - 1),
                            perf_mode=DRS)

            outq = [nc.sync, nc.gpsimd]
            qi = 0

            def store(ot, t, coff, width, last=False):
                nonlocal qi
                # per-512-block DMAs into the tile-major (contiguous) out;
                # the narrow (triangle) tiles target a JB-block suffix.
                for s in range(-(-width // JB)):
                    a, b = s * JB, min((s + 1) * JB, width)
                    w = (coff + a) // JB
                    cl = (coff + a) % JB
                    if last and s == 1:
                        q = nc.scalar
                    else:
                        q = outq[qi % 2]
                        qi += 1
                    q.dma_start(out_d[t, w, :, cl:cl + (b - a)], ot[:, a:b])

            # narrow group: diagonal block upper triangle, PSUM->ACT direct
            for t in range(NT):
                width = JB - t * P
                gp = g_pool.tile([P, width], f32, name=f"gpn_{t}", tag="g")
                mms(gp, t, t * P, width)
                ot = ot_pool.tile([P, width], bf16, name=f"otn_{t}",
                                  tag="ot", bufs=8)
                nc.scalar.activation(ot[:], gp[:], ACTF.Exp,
                                     bias=stat_sb[:, t:t + 1],
                                     scale=stat_sb[:, NT:NT + 1])
                store(ot, t, t * P, width)

            # wide groups: PSUM -> ACT Exp -> bf16 SBUF -> DMA
            ngrp = len(WIDE) * NT
            for gi, (coff, width) in enumerate(WIDE):
                for t in range(NT):
                    gp = g_pool.tile([P, width], f32,
                                     name=f"gp_{t}_{coff}", tag="g")
                    mms(gp, t, coff, width)
                    ot = ot_pool.tile([P, width], bf16,
                                      name=f"ot_{t}_{coff}", tag="ot", bufs=8)
                    nc.scalar.activation(ot[:], gp[:], ACTF.Exp,
                                         bias=stat_sb[:, t:t + 1],
                                         scale=stat_sb[:, NT:NT + 1])
                    store(ot, t, coff, width,
                          last=(gi * NT + t == ngrp - 1))

    _split_waits(nc)
    return nc


_NC = None


def _dr_layout(a):
    """[512, M] -> [KT, P, 2, M] DoubleRow plane layout (d = kt*256+i*128+p)."""
    return np.ascontiguousarray(a.reshape(KT, 2, P, a.shape[1])
                                .transpose(0, 2, 1, 3))


def _swi_layout(a):
    """[512, M] -> [KT, P, M, 2] DoubleRowSwInterleave weights: per k-tile,
    (plane0, plane1) pairs adjacent along the last axis, with the m index
    reversed inside each 128-wide stationary slice. Logical dim 480
    (kt=1, p=96, plane=1) is the augmentation row: its weight is 1 and the
    host writes q_j into the matching wj slot."""
    m = a.shape[1]
    w = a.reshape(KT, 2, P, m).transpose(0, 2, 3, 1)    # [kt, p, m, i]
    w = w.reshape(KT, P, m // P, P, 2)[:, :, :, ::-1, :]
    w = np.ascontiguousarray(w.reshape(KT, P, m, 2))
    w[1, 96, :, 1] = 1.0
    return w


def make_in_maps(X):
    import ml_dtypes
    Xf = np.asarray(X, dtype=np.float64)
    X8 = np.asarray(X, dtype=ml_dtypes.float8_e4m3)
    XT8 = np.ascontiguousarray(X8.T)              # [512, 4096]

    # host-side stats: sq of the QUANTIZED points (what the GEMM computes),
    # sigma^2 of the ORIGINAL points (the reference's divisor).
    sq = (XT8.astype(np.float64) ** 2).sum(axis=0)          # [4096]
    mu = Xf.mean(axis=0)
    sigma2 = 2.0 * ((Xf ** 2).sum(axis=1).mean() - mu @ mu)  # mean(d2), exact
    q8 = np.asarray(-(sq - 512.0) / 2.0, dtype=ml_dtypes.float8_e4m3)
    scale = 1.0 / sigma2
    bias = -(sq + 512.0) / (2.0 * sigma2)                    # [4096]

    maps = []
    for c in range(NCORES):
        lo = c * RPC
        idx = (lo + np.arange(WIN)) % N
        wjc = _dr_layout(XT8[:, idx])
        wjc[1, 96, 1, :] = q8[idx]          # aug row: q_j rides the matmul
        # [KT, P, 2, WIN] viewed as [KT, P, 2, W, JB] (already contiguous)
        wjc = wjc.reshape(KT, P, 2, W, JB)
        xt = _swi_layout(XT8[:, lo:lo + RPC])   # [KT, P, RPC, 2]
        xt = np.ascontiguousarray(xt.transpose(1, 0, 2, 3))
        stat = np.empty((P, NT + 1), dtype=np.float32)
        stat[:, :NT] = bias[lo:lo + RPC].reshape(NT, P).T
        stat[:, NT] = scale
        maps.append({"wj": wjc, "xtc": xt, "stat": stat})
    return maps


def assemble(slabs):
    """slabs: per-core [NT, W, P, JB] (bf16) -> full [N, N] f32."""
    out = np.empty((N, N), dtype=np.float32)
    for c in range(NCORES):
        lo = c * RPC
        slab = np.asarray(slabs[c], dtype=np.float32)
        slab = slab.transpose(0, 2, 1, 3).reshape(RPC, WIN)
        n1 = min(WIN, N - lo)
        out[lo:lo + RPC, lo:lo + n1] = slab[:, :n1]
        if n1 < WIN:
            out[lo:lo + RPC, :WIN - n1] = slab[:, n1:]
    # mirror block-distance {5,6,7} from their transposed {3,2,1} partners
    for bi in range(NCORES):
        for dd in (5, 6, 7):
            bj = (bi + dd) % NCORES
            out[bi * RPC:(bi + 1) * RPC, bj * RPC:(bj + 1) * RPC] = \
                out[bj * RPC:(bj + 1) * RPC, bi * RPC:(bi + 1) * RPC].T
    # mirror the sub-triangle of each diagonal block (128-row granularity)
    for c in range(NCORES):
        lo = c * RPC
        B = out[lo:lo + RPC, lo:lo + RPC]
        for t in range(1, NT):
            tp = t * P
            B[tp:tp + P, :tp] = B[:tp, tp:tp + P].T
    return out


def kernel(X: np.ndarray) -> np.ndarray:
    global _NC
    if _NC is None:
        _NC = build()
    res = run_bass_kernel_spmd(_NC, make_in_maps(X),
                               list(range(NCORES))).results
    return assemble([res[c]["out"] for c in range(NCORES)])


# revision 33
# speedup vs baseline: 1.0916x; 1.0916x over previous
"""Gaussian kernel matrix on 8 Trainium2 NeuronCores — host-stats fp8 GEMM.

out = exp(-d2 / (2*sigma^2)),  d2[i,j] = ||x_i||^2 + ||x_j||^2 - 2 x_i.x_j,
sigma^2 = mean(d2) = 2*(mean(sq) - ||mean(X)||^2).

Strategy v4:
- Symmetry: core c computes rows [c*512,(c+1)*512) x a wrapped column window
  of 2560 cols starting at c*512 (5 of 8 j-blocks); the host mirrors the
  remaining blocks by transposition. The diagonal j-block is computed only
  for cols >= t*128 per row-tile (upper triangle at 128-granularity); the
  host mirrors the rest.
- GEMM in fp8 e4m3 DoubleRowSwInterleave (K=256/matmul); X quantized on the
  host, kernel computes the exact Gaussian kernel of the quantized points.
- ALL statistics on the host (O(N*D)): sq_j, exact sigma^2 from unquantized
  X, q_j = -(sq_j-512)/2 baked into the sacrificed contraction row (logical
  dim 480) of wj; bias_i/scale shipped as a [P,5] f32 input consumed by the
  fused ACT epilogue out = Exp(scale*G + bias_i).
- PE p-state: max clock needs ~3us of continuous PE busy; idle gaps reset
  it. Warmup matmuls over raw (uninitialized) SBUF bridge from the PE
  preamble until the first wj chunk lands.
- Epilogue: narrow (triangle) group PSUM->ACT first for an early start,
  then 8 wide [128,1024] 2-bank PSUM tiles, each one Exp pass (amortizes
  the ~250ns fixed ACT PSUM-access cost).
- DMA: all DRAM tensors laid out so every transfer is contiguous in DRAM;
  few large transfers (each issue costs ~0.65us of engine time), placed in
  consumption order across the three DMA queues (SP/sync,
  Activation/scalar, Pool/gpsimd); outputs alternate sync/gpsimd with the
  final tile split onto sync+scalar (scalar is free after its last ACT).
"""
import numpy as np
import sys

sys.path.insert(0, "/opt/trn_rl_repo")
from concourse import bass, tile, mybir  # noqa: E402
from concourse.bass_utils import run_bass_kernel_spmd  # noqa: E402

N, D, NCORES = 4096, 512, 8
RPC = 512                  # output rows per core
P = 128                    # partitions
KT = 2                     # DoubleRow k-tiles (256 contraction rows each)
JB = 512                   # j-block width
W = 5                      # window j-blocks per core
WIN = W * JB               # 2560 window columns
NT = RPC // P              # 4 row-tiles per core
NWARM = 19                 # PE warmup matmuls (~215ns each)
f32 = mybir.dt.float32
f16 = mybir.dt.float16
bf16 = mybir.dt.bfloat16
fp8 = mybir.dt.float8e4
ACTF = mybir.ActivationFunctionType
DR = mybir.MatmulPerfMode.DoubleRow
DRS = mybir.MatmulPerfMode.DoubleRowSwInterleave

# wide groups: (window col offset, width); each pairs 2 j-blocks into a
# 2-bank PSUM tile. The diagonal block (cols 0:512) is the narrow group.
WIDE = [(512, 1024), (1536, 1024)]


def _split_waits(nc, max_waits=1):
    """walrus in this image encodes at most one sync-wait per instruction;
    split extras into single-wait NOPs placed just before the instruction."""
    for fn in nc.m.functions:
        for bb in fn.blocks:
            out = []
            for inst in bb.instructions:
                si = inst.sync_info
                if si and si.on_wait and len(si.on_wait) > max_waits:
                    waits = list(si.on_wait)
                    extra, keep = waits[:-max_waits], waits[-max_waits:]
                    for j, w in enumerate(extra):
                        out.append(mybir.InstNoOp(
                            name=f"{inst.name}-ws{j}", engine=inst.engine,
                            sync_info=mybir.SyncInfo(on_wait=[w], on_update=[])))
                    si.on_wait = keep
                out.append(inst)
            bb.instructions = out


def build():
    nc = bass.Bass()
    wj_in = nc.dram_tensor("wj", [KT, P, 2, W, JB], fp8, kind="ExternalInput")
    xk1_in = nc.dram_tensor("xk1", [P, 2, RPC], fp8, kind="ExternalInput")
    stat_in = nc.dram_tensor("stat", [P, NT + 1], f32, kind="ExternalInput")
    out_d = nc.dram_tensor("out", [NT, W, P, JB], bf16, kind="ExternalOutput")

    with tile.TileContext(nc) as tc:
        with (
            tc.tile_pool(name="xt", bufs=1) as xt_pool,
            tc.tile_pool(name="ot", bufs=1) as ot_pool,
            tc.tile_pool(name="small", bufs=1) as small_pool,
            tc.tile_pool(name="g", bufs=3, space="PSUM") as g_pool,
            tc.tile_pool(name="wup", bufs=1, space="PSUM") as wup_pool,
        ):
            xk1 = xt_pool.tile([P, 2, RPC], fp8, name="xk1", tag="xk1")
            wj = [xt_pool.tile([P, 2, W, JB], fp8, name=f"wj{k}", tag=f"wj{k}")
                  for k in range(KT)]
            stat_sb = small_pool.tile([P, NT + 1], f32, tag="stat")

            def wjd(q, k, w0, w1):
                q.dma_start(wj[k][:, :, w0:w1, :], wj_in[k][:, :, w0:w1, :])

            # Transfers in consumption order across the three queues (HBM
            # read BW ~360 GB/s shared, so the concurrent set at any time
            # should be the next-needed chunks). DMA completion semaphores
            # cost ~1us, queues spin up ~1.2us after the first issue, the
            # gpsimd (SWDGE) queue starts ~1.5us late, and the scalar
            # queue starves (~30GB/s) once the other two are running — so
            # scalar gets the k0-critical first chunk + small/late items.
            # k=0's stationary is a VIEW of wj w0, so the k0 path needs
            # only one chunk; k1 needs xk1 + wj1w0 (both on sync).
            wjd(nc.scalar, 0, 0, 1)
            nc.scalar.dma_start(stat_sb[:], stat_in[:, :])
            wjd(nc.scalar, 0, 3, 4)

            nc.sync.dma_start(xk1[:], xk1_in[:, :])
            wjd(nc.sync, 1, 0, 1)
            wjd(nc.sync, 0, 1, 2)
            wjd(nc.sync, 1, 1, 2)

            wjd(nc.gpsimd, 0, 2, 3)
            wjd(nc.gpsimd, 1, 2, 3)
            wjd(nc.gpsimd, 1, 3, 4)
            wjd(nc.gpsimd, 0, 4, 5)
            wjd(nc.gpsimd, 1, 4, 5)

            # Exp ACT table load (~1.3us) during DMA dead time.
            warm_f = small_pool.tile([1, 16], f32, tag="warm_f")
            nc.vector.memset(warm_f[:], 1.0)
            warm_sb = small_pool.tile([1, 16], f32, tag="warm_sb")
            nc.scalar.activation(warm_sb[:], warm_f[:], ACTF.Exp)

            # PE p-state warmup over raw SBUF (no deps, garbage values into
            # a scratch PSUM bank).
            wstat = nc.alloc_sbuf_tensor("wup_stat", [P, 2, P], fp8).ap()
            wmov = nc.alloc_sbuf_tensor("wup_mov", [P, 2, 256], fp8).ap()
            wp = wup_pool.tile([P, 256], f32, tag="wup")
            for i in range(NWARM):
                nc.tensor.matmul(wp[:], wstat, wmov,
                                 start=True, stop=True, perf_mode=DR)

            def mms(gp, t, coff, width):
                ts = slice(t * P, (t + 1) * P)
                # plain-DoubleRow stationaries, plane-major [P, 2, m]:
                # k=0 is a view of the wj w0 chunk; k=1 is the small xk1
                # input whose logical dim 480 (p=96, plane 1) is 1.0 so
                # the q_j aug row in wj rides the k=1 matmul.
                lhs = [wj[0][:, :, 0, ts], xk1[:, :, ts]]
                for s in range(-(-width // JB)):
                    a, b = s * JB, min((s + 1) * JB, width)
                    w, cl = (coff + a) // JB, (coff + a) % JB
                    for k in range(KT):
                        nc.tensor.matmul(
                            gp[:, a:b], lhs[k],
                            wj[k][:, :, w, cl:cl + (b - a)],
                            start=(k == 0), stop=(k == KT - 1),
                            perf_mode=DR)

            outq = [nc.sync, nc.gpsimd]
            qi = 0

            def store(ot, t, coff, width, last=False):
                nonlocal qi
                # per-512-block DMAs into the tile-major (contiguous) out;
                # the narrow (triangle) tiles target a JB-block suffix.
                for s in range(-(-width // JB)):
                    a, b = s * JB, min((s + 1) * JB, width)
                    w = (coff + a) // JB
                    cl = (coff + a) % JB
                    if last and s == 1:
                        q = nc.scalar
                    else:
                        q = outq[qi % 2]
                        qi += 1
                    q.dma_start(out_d[t, w, :, cl:cl + (b - a)], ot[:, a:b])

            # narrow group: diagonal block upper triangle, PSUM->ACT direct
            for t in range(NT):
                width = JB - t * P
                gp = g_pool.tile([P, width], f32, name=f"gpn_{t}", tag="g")
                mms(gp, t, t * P, width)
                ot = ot_pool.tile([P, width], bf16, name=f"otn_{t}",
                                  tag="ot", bufs=8)
                nc.scalar.activation(ot[:], gp[:], ACTF.Exp,
                                     bias=stat_sb[:, t:t + 1],
                                     scale=stat_sb[:, NT:NT + 1])
                store(ot, t, t * P, width)

            # wide groups: PSUM -> ACT Exp -> bf16 SBUF -> DMA
            ngrp = len(WIDE) * NT
            for gi, (coff, width) in enumerate(WIDE):
                for t in range(NT):
                    gp = g_pool.tile([P, width], f32,
                                     name=f"gp_{t}_{coff}", tag="g")
                    mms(gp, t, coff, width)
                    ot = ot_pool.tile([P, width], bf16,
                                      name=f"ot_{t}_{coff}", tag="ot", bufs=8)
                    nc.scalar.activation(ot[:], gp[:], ACTF.Exp,
                                         bias=stat_sb[:, t:t + 1],
                                         scale=stat_sb[:, NT:NT + 1])
                    store(ot, t, coff, width,
                          last=(gi * NT + t == ngrp - 1))

    _split_waits(nc)
    return nc


_NC = None


def _dr_layout(a):
    """[512, M] -> [KT, P, 2, M] DoubleRow plane layout (d = kt*256+i*128+p)."""
    return np.ascontiguousarray(a.reshape(KT, 2, P, a.shape[1])
                                .transpose(0, 2, 1, 3))


def _xk1_layout(a):
    """own-block [512, RPC] -> k=1 plane-major stationary [P, 2, RPC]:
    xk1[p, i, m] = a[256 + i*128 + p, m]. Logical dim 480 (p=96, plane 1)
    is the augmentation row: its weight is 1 so the q_j row baked into wj
    rides the k=1 matmul."""
    w = np.ascontiguousarray(
        a[2 * P:].reshape(2, P, a.shape[1]).transpose(1, 0, 2))
    w[96, 1, :] = 1.0
    return w


def make_in_maps(X):
    import ml_dtypes
    Xf = np.asarray(X, dtype=np.float64)
    X8 = np.asarray(X, dtype=ml_dtypes.float8_e4m3)
    XT8 = np.ascontiguousarray(X8.T)              # [512, 4096]

    # host-side stats: sq of the QUANTIZED points (what the GEMM computes),
    # sigma^2 of the ORIGINAL points (the reference's divisor).
    sq = (XT8.astype(np.float64) ** 2).sum(axis=0)          # [4096]
    mu = Xf.mean(axis=0)
    sigma2 = 2.0 * ((Xf ** 2).sum(axis=1).mean() - mu @ mu)  # mean(d2), exact
    q8 = np.asarray(-(sq - 512.0) / 2.0, dtype=ml_dtypes.float8_e4m3)
    scale = 1.0 / sigma2
    bias = -(sq + 512.0) / (2.0 * sigma2)                    # [4096]

    maps = []
    for c in range(NCORES):
        lo = c * RPC
        idx = (lo + np.arange(WIN)) % N
        wjc = _dr_layout(XT8[:, idx])
        wjc[1, 96, 1, :] = q8[idx]          # aug row: q_j rides the matmul
        # [KT, P, 2, WIN] viewed as [KT, P, 2, W, JB] (already contiguous)
        wjc = wjc.reshape(KT, P, 2, W, JB)
        xk1 = _xk1_layout(XT8[:, lo:lo + RPC])
        stat = np.empty((P, NT + 1), dtype=np.float32)
        stat[:, :NT] = bias[lo:lo + RPC].reshape(NT, P).T
        stat[:, NT] = scale
        maps.append({"wj": wjc, "xk1": xk1, "stat": stat})
    return maps


def assemble(slabs):
    """slabs: per-core [NT, W, P, JB] (bf16) -> full [N, N] f32."""
    out = np.empty((N, N), dtype=np.float32)
    for c in range(NCORES):
        lo = c * RPC
        slab = np.asarray(slabs[c], dtype=np.float32)
        slab = slab.transpose(0, 2, 1, 3).reshape(RPC, WIN)
        n1 = min(WIN, N - lo)
        out[lo:lo + RPC, lo:lo + n1] = slab[:, :n1]
        if n1 < WIN:
            out[lo:lo + RPC, :WIN - n1] = slab[:, n1:]
    # mirror block-distance {5,6,7} from their transposed {3,2,1} partners
    for bi in range(NCORES):
        for dd in (5, 6, 7):
            bj = (bi + dd) % NCORES
            out[bi * RPC:(bi + 1) * RPC, bj * RPC:(bj + 1) * RPC] = \
                out[bj * RPC:(bj + 1) * RPC, bi * RPC:(bi + 1) * RPC].T
    # mirror the sub-triangle of each diagonal block (128-row granularity)
    for c in range(NCORES):
        lo = c * RPC
        B = out[lo:lo + RPC, lo:lo + RPC]
        for t in range(1, NT):
            tp = t * P
            B[tp:tp + P, :tp] = B[:tp, tp:tp + P].T
    return out


def kernel(X: np.ndarray) -> np.ndarray:
    global _NC
    if _NC is None:
        _NC = build()
    res = run_bass_kernel_spmd(_NC, make_in_maps(X),
                               list(range(NCORES))).results
    return assemble([res[c]["out"] for c in range(NCORES)])
